# revision 38
# baseline (speedup 1.0000x reference)
"""Distributed 3-layer GAT encoder on 8 TRN2 NeuronCores (Bass/Tile).

Strategy (graph partition by dst, per the sharding hint):
  - Core c owns dst nodes [2500c, 2500c+2500), padded to 2560 = 20 blocks x 128.
  - Per layer, a full node table lives in each core's HBM:
      tab_l [20480, 320|128] fp16 : rows [h | alpha_src | pad]
    built by matmuls from all-gathered transposed features with folded
    weights [W_l | W_l.a_src]; alpha_dst lives in a per-core local table
    adloc_l [2560, H] fp16 written at the previous layer's flush.
  - Edge phase (per 128-dst block, edges dst-sorted in tiles of 128):
      one dma_gather of [h|alpha_src] rows by src;
      indicator ind[e,d] and its transpose built by DVE compares vs iota;
      alpha_dst expanded per edge via matmul(lhsT=indT, rhs=adloc block);
      p = exp(leaky_relu(as+ad)) (no segment-max: verified safe);
      numerator + denominator accumulated in PSUM via matmuls
      (lhsT=ind, rhs=[p*h] and rhs=[p]).
  - Flush: normalize, mean over heads, bias, relu -> PE transpose ->
    AllGather fp16 -> next layer table rebuild.
"""
import numpy as np

N = 20000
NCORES = 8
NPC = 2500
NPAD = 2560
NBLK = 20
NTOT = NCORES * NPAD  # 20480
P = 128

LAST_RESULT = None


# ----------------------------------------------------------------- host prep
def _wrap16(idx, ncols):
    n = len(idx)
    w = np.zeros((P, ncols), dtype=np.int16)
    cols = (n + 15) // 16
    assert cols <= ncols
    buf = np.zeros((16, cols), dtype=np.int16)
    buf[np.arange(n) % 16, np.arange(n) // 16] = idx
    for g in range(8):
        w[16 * g:16 * g + 16, :cols] = buf
    return w


def _preprocess(edge_index):
    src = np.asarray(edge_index[0], dtype=np.int64)
    dst = np.asarray(edge_index[1], dtype=np.int64)
    loop = np.arange(N, dtype=np.int64)
    src = np.concatenate([src, loop])
    dst = np.concatenate([dst, loop])

    own_s = src // NPC
    src_p = own_s * NPAD + (src - own_s * NPC)
    own = dst // NPC
    dst_loc = dst - own * NPC

    order = np.lexsort((dst_loc, own))
    src_p, dst_loc, own = src_p[order], dst_loc[order], own[order]
    blk = dst_loc // P
    counts = np.zeros((NCORES, NBLK), dtype=np.int64)
    for c in range(NCORES):
        for b in range(NBLK):
            counts[c, b] = np.sum((own == c) & (blk == b))
    T = np.maximum(1, np.ceil(counts.max(axis=0) / P).astype(np.int64))
    Ttot = int(T.sum())

    wrap_src = np.zeros((NCORES, P, Ttot * 8), dtype=np.int16)
    dstloc16 = np.full((NCORES, P, Ttot), -1.0, dtype=np.float16)
    dlocrep = np.full((NCORES, Ttot * P), -1.0, dtype=np.float16)
    dlocprep = np.full((NCORES, P, Ttot * P), -1.0, dtype=np.float16)
    off8 = np.zeros(NBLK + 1, dtype=np.int64)
    offT = np.zeros(NBLK + 1, dtype=np.int64)
    for b in range(NBLK):
        off8[b + 1] = off8[b] + T[b] * 8
        offT[b + 1] = offT[b] + T[b]
    for c in range(NCORES):
        m_c = own == c
        for b in range(NBLK):
            m = m_c & (blk == b)
            cnt = int(counts[c, b])
            nb = int(T[b]) * P
            isrc = np.zeros(nb, dtype=np.int64)
            isrc[:cnt] = src_p[m]
            dl = np.full(nb, -1.0, dtype=np.float32)
            dl[:cnt] = dst_loc[m] - b * P
            wrap_src[c, :, off8[b]:off8[b + 1]] = _wrap16(isrc, int(T[b]) * 8)
            dstloc16[c, :, offT[b]:offT[b + 1]] = (
                dl.reshape(int(T[b]), P).T.astype(np.float16))
            dlocrep[c, offT[b] * P:offT[b + 1] * P] = dl.astype(np.float16)
            dlocprep[c, :, offT[b] * P:offT[b + 1] * P] = np.repeat(
                dl.reshape(int(T[b]), P).T.astype(np.float16), P, axis=1)
    # replicate along partitions: [NCORES, P, Ttot*P]
    dlocrep = np.repeat(dlocrep[:, None, :], P, axis=1)
    return T, off8, offT, wrap_src, dstloc16, dlocrep, dlocprep


# ------------------------------------------------------------- build program
def _build(T, off8, offT, do_compile=True):
    from concourse import bass, bacc, mybir, tile

    f16 = mybir.dt.float16
    f32 = mybir.dt.float32
    i16 = mybir.dt.int16
    AF = mybir.ActivationFunctionType
    OP = mybir.AluOpType

    Ttot = int(T.sum())
    NW = Ttot * 8
    NVALID_LAST = NPC - (NBLK - 1) * P  # 68

    nc = bacc.Bacc("TRN2", target_bir_lowering=False, debug=False,
                   num_devices=NCORES)

    # inputs
    xT16 = nc.dram_tensor("xT16", [P, NTOT], f16, kind="ExternalInput")
    xlocT = nc.dram_tensor("xlocT", [P, NPAD], f16, kind="ExternalInput")
    iwsrc = nc.dram_tensor("iwsrc", [P, NW], i16, kind="ExternalInput")
    dloc = nc.dram_tensor("dloc", [P, Ttot], f16, kind="ExternalInput")
    dlocrep = nc.dram_tensor("dlocrep", [P, Ttot * P], f16,
                             kind="ExternalInput")
    dlocprep = nc.dram_tensor("dlocprep", [P, Ttot * P], f16,
                              kind="ExternalInput")
    iotacr = nc.dram_tensor("iotacr", [P, P], f16, kind="ExternalInput")
    iotabig = nc.dram_tensor("iotabig", [P, int(T.max()) * P], f16,
                             kind="ExternalInput")
    iotacrbig = nc.dram_tensor("iotacrbig", [P, int(T.max()) * P], f16,
                               kind="ExternalInput")
    c100 = nc.dram_tensor("c100", [P, 32], f32, kind="ExternalInput")
    c1em8 = nc.dram_tensor("c1em8", [P, 32], f32, kind="ExternalInput")
    iota = nc.dram_tensor("iota", [P, P], f16, kind="ExternalInput")
    iotac = nc.dram_tensor("iotac", [P, 1], f16, kind="ExternalInput")
    ident16 = nc.dram_tensor("ident16", [P, P], f16, kind="ExternalInput")
    identf = nc.dram_tensor("identf", [P, P], f32, kind="ExternalInput")
    w1c = nc.dram_tensor("w1c", [128, 260], f16, kind="ExternalInput")
    w2c = nc.dram_tensor("w2c", [64, 260], f16, kind="ExternalInput")
    w3c = nc.dram_tensor("w3c", [64, 33], f16, kind="ExternalInput")
    wad1 = nc.dram_tensor("wad1", [128, 4], f16, kind="ExternalInput")
    wad2 = nc.dram_tensor("wad2", [64, 4], f16, kind="ExternalInput")
    wad3 = nc.dram_tensor("wad3", [64, 1], f16, kind="ExternalInput")
    b1r = nc.dram_tensor("b1r", [P, 64], f32, kind="ExternalInput")
    b2r = nc.dram_tensor("b2r", [P, 64], f32, kind="ExternalInput")
    b3r = nc.dram_tensor("b3r", [P, 32], f32, kind="ExternalInput")
    bmr = nc.dram_tensor("bmr", [P, 32], f32, kind="ExternalInput")
    bvr = nc.dram_tensor("bvr", [P, 32], f32, kind="ExternalInput")
    wm = nc.dram_tensor("wm", [32, 32], f32, kind="ExternalInput")
    wv = nc.dram_tensor("wv", [32, 32], f32, kind="ExternalInput")

    # outputs
    z_out = nc.dram_tensor("z", [NPC, 32], f32, kind="ExternalOutput")
    zm_out = nc.dram_tensor("zmean", [NPC, 32], f32, kind="ExternalOutput")
    zv_out = nc.dram_tensor("zvar", [NPC, 32], f32, kind="ExternalOutput")

    with tile.TileContext(nc) as tc:
        with (
            tc.tile_pool(name="const", bufs=1) as cpool,
            tc.tile_pool(name="sb", bufs=3) as sb,
            tc.tile_pool(name="gth", bufs=5) as gth,
            tc.tile_pool(name="blk", bufs=3) as blk,
            tc.tile_pool(name="blks", bufs=3) as blks,
            tc.tile_pool(name="psreb", bufs=2, space="PSUM") as psreb,
            tc.tile_pool(name="psad", bufs=2, space="PSUM") as psad,
            tc.tile_pool(name="pssm", bufs=1, space="PSUM") as pssm,
            tc.tile_pool(name="psagg", bufs=3, space="PSUM") as psagg,
            tc.tile_pool(name="dram", bufs=1, space="DRAM") as dram,
        ):
            tab1 = dram.tile([NTOT, 384], f16)
            tab2 = dram.tile([NTOT, 384], f16)
            tab3 = dram.tile([NTOT, 128], f16)
            adloc1 = dram.tile([NPAD, 4], f16)
            adloc2 = dram.tile([NPAD, 4], f16)
            adloc3 = dram.tile([NPAD, 1], f16)
            x2T_loc = dram.tile([64, NPAD], f16)
            x3T_loc = dram.tile([64, NPAD], f16)
            x2T_full = dram.tile([NCORES, 64, NPAD], f16)
            x3T_full = dram.tile([NCORES, 64, NPAD], f16)

            def ld(shape, dt, src):
                t = cpool.tile(shape, dt, tag="c_" + src.name)
                nc.sync.dma_start(out=t[:], in_=src[:, :])
                return t

            id16_sb = ld([P, P], f16, ident16)
            idf_sb = ld([P, P], f32, identf)
            w1c_sb = ld([128, 260], f16, w1c)
            w2c_sb = ld([64, 260], f16, w2c)
            w3c_sb = ld([64, 33], f16, w3c)
            wad1_sb = ld([128, 4], f16, wad1)
            wad2_sb = ld([64, 4], f16, wad2)
            wad3_sb = ld([64, 1], f16, wad3)
            b1r_sb = ld([P, 64], f32, b1r)
            b2r_sb = ld([P, 64], f32, b2r)
            b3r_sb = ld([P, 32], f32, b3r)
            bmr_sb = ld([P, 32], f32, bmr)
            bvr_sb = ld([P, 32], f32, bvr)
            wm_sb = ld([32, 32], f32, wm)
            wv_sb = ld([32, 32], f32, wv)
            iwsrc_sb = ld([P, NW], i16, iwsrc)
            iotabig_sb = ld([P, int(T.max()) * P], f16, iotabig)
            iotacrbig_sb = ld([P, int(T.max()) * P], f16, iotacrbig)
            c100_sb = ld([P, 32], f32, c100)
            c1em8_sb = ld([P, 32], f32, c1em8)
            xloc_sb = ld([P, NPAD], f16, xlocT)

            # -------- table rebuild: tab rows = fp16(xT^T @ wc) ----------
            G = 4 if NBLK % 4 == 0 else 2
            def rebuild(src_getter4, wc_sb, in_c, ncols, tab):
                for t4 in range(NTOT // P // G):
                    e1 = nc.sync if t4 % 2 == 0 else nc.scalar
                    e2 = nc.scalar if t4 % 2 == 0 else nc.sync
                    lh = sb.tile([in_c, G * P], f16, tag="reblh")
                    e1.dma_start(out=lh[:], in_=src_getter4(t4))
                    h16 = sb.tile([P, G, ncols], f16, tag="rebh")
                    for j in range(G):
                        pr = psreb.tile([P, ncols], f32, space="PSUM",
                                        tag="reb")
                        nc.tensor.matmul(
                            out=pr[:], lhsT=lh[:, j * P:(j + 1) * P],
                            rhs=wc_sb[:in_c, :ncols], start=True, stop=True)
                        if j % 2 == 0:
                            nc.vector.tensor_copy(out=h16[:, j, :], in_=pr[:])
                        else:
                            nc.scalar.activation(h16[:, j, :], pr[:], AF.Copy)
                    e2.dma_start(
                        out=tab[t4 * G * P:(t4 + 1) * G * P, 0:ncols]
                        .rearrange("(j r) c -> r j c", j=G),
                        in_=h16[:])

            # -------- alpha_dst local table ------------------------------
            def write_adloc(xt_sb_blk, wad_sb, in_c, H, adloc, b):
                pad = pssm.tile([P, 4], f32, space="PSUM", tag="sm")
                nc.tensor.matmul(out=pad[:, :H], lhsT=xt_sb_blk,
                                 rhs=wad_sb[:in_c, :H], start=True, stop=True)
                a16 = sb.tile([P, H], f16, tag="a16")
                nc.scalar.activation(a16[:], pad[:, :H], AF.Copy)
                nc.scalar.dma_start(out=adloc[b * P:(b + 1) * P, :],
                                    in_=a16[:])

            # -------- edge phase ------------------------------------------
            def edge_layer(tab, adloc, elem, H, C, flush):
                HC = H * C
                for b in range(NBLK):
                    Tb = int(T[b])
                    nidx = Tb * P
                    g = gth.tile([P, Tb, elem], f16, tag="g")
                    nc.gpsimd.dma_gather(
                        out_ap=g[:], in_ap=tab[:, :],
                        idxs_ap=iwsrc_sb[:, int(off8[b]):int(off8[b]) + Tb * 8],
                        num_idxs=nidx, num_idxs_reg=nidx, elem_size=elem,
                        elem_step=int(tab.shape[1]),
                        single_packet=nidx <= 1024)
                    dlr = blks.tile([P, Tb * P], f16, tag="dlr")
                    nc.sync.dma_start(
                        out=dlr[:],
                        in_=dlocrep[:, int(offT[b]) * P:int(offT[b + 1]) * P])
                    dlp = blks.tile([P, Tb * P], f16, tag="dlp")
                    nc.sync.dma_start(
                        out=dlp[:],
                        in_=dlocprep[:, int(offT[b]) * P:int(offT[b + 1]) * P])
                    adb = sb.tile([P, H], f16, tag="adb")
                    nc.sync.dma_start(out=adb[:],
                                      in_=adloc[b * P:(b + 1) * P, :])

                    pa = psagg.tile([P, HC + H], f32, space="PSUM", tag="agg")
                    pad_all = psad.tile([P, Tb, H], f32, space="PSUM",
                                        tag="ad")
                    indT_all = blks.tile([P, Tb, P], f16, tag="indT")
                    nc.vector.tensor_tensor(
                        out=indT_all[:].rearrange("p t q -> p (t q)"),
                        in0=iotacrbig_sb[:, :Tb * P],
                        in1=dlr[:], op=OP.is_equal)
                    for t in range(Tb):
                        nc.tensor.matmul(out=pad_all[:, t, :],
                                         lhsT=indT_all[:, t, :],
                                         rhs=adb[:], start=True, stop=True)
                    ESW = elem // C
                    es = sb.tile([P, Tb, ESW], f32, tag="es")
                    if ESW > H:
                        nc.vector.memset(es[:, :, H:ESW], 0.0)
                    nc.vector.tensor_add(out=es[:, :, 0:H],
                                         in0=g[:, :, HC:HC + H],
                                         in1=pad_all[:])
                    es2 = sb.tile([P, Tb, ESW], f32, tag="es2")
                    nc.vector.tensor_scalar_mul(out=es2[:], in0=es[:],
                                                scalar1=0.2)
                    nc.vector.tensor_max(out=es[:], in0=es[:], in1=es2[:])

                    ind_all = blks.tile([P, Tb, P], f16, tag="indA")
                    nc.vector.tensor_tensor(
                        out=ind_all[:].rearrange("p t q -> p (t q)"),
                        in0=dlp[:], in1=iotabig_sb[:, :Tb * P],
                        op=OP.is_equal)
                    pex_all = blk.tile([P, Tb, elem], f16, tag="pex")
                    nc.scalar.activation(
                        pex_all[:].rearrange("p t (w c) -> p t w c", w=ESW),
                        es[:, :, :, None].to_broadcast([P, Tb, ESW, C]),
                        AF.Exp)
                    nc.vector.tensor_mul(out=pex_all[:], in0=g[:],
                                         in1=pex_all[:])
                    nc.scalar.activation(
                        pex_all[:, :, HC:HC + H], es[:, :, 0:H], AF.Exp)
                    for t in range(Tb):
                        nc.tensor.matmul(
                            out=pa[:], lhsT=ind_all[:, t, :],
                            rhs=pex_all[:, t, 0:HC + H],
                            start=(t == 0), stop=(t == Tb - 1))
                    flush(b, pa)

            # -------- flush -----------------------------------------------
            def flush_12(b, pa, H, C, brep_sb, xT_loc_dram, wadn_sb, adlocn,
                         Hn):
                HC = H * C
                inv = sb.tile([P, H], f32, tag="inv")
                nc.vector.tensor_scalar_add(out=inv[:], in0=pa[:, HC:HC + H],
                                            scalar1=1e-16)
                nc.vector.reciprocal(out=inv[:], in_=inv[:])
                nc.vector.tensor_scalar_mul(out=inv[:], in0=inv[:],
                                            scalar1=1.0 / H)
                ivx = sb.tile([P, HC], f32, tag="ivx")
                nc.scalar.activation(
                    ivx[:].rearrange("p (h c) -> p h c", h=H),
                    inv[:, :, None].to_broadcast([P, H, C]), AF.Copy)
                nrm = sb.tile([P, HC], f32, tag="nrm")
                nc.vector.tensor_mul(out=nrm[:], in0=pa[:, 0:HC], in1=ivx[:])
                m = sb.tile([P, C], f32, tag="mean")
                nc.vector.tensor_reduce(
                    out=m[:], in_=nrm[:].rearrange("p (h c) -> p c h", h=H),
                    axis=mybir.AxisListType.X, op=OP.add)
                nc.vector.tensor_add(out=m[:], in0=m[:], in1=brep_sb[:, :C])
                x16 = sb.tile([P, C], f16, tag="x16")
                nc.scalar.activation(x16[:], m[:], AF.Relu)
                pt = pssm.tile([C, P], f16, space="PSUM", tag="sm")
                nc.tensor.transpose(out=pt[:], in_=x16[:], identity=id16_sb[:])
                xt = sb.tile([C, P], f16, tag="xt")
                nc.scalar.activation(xt[:], pt[:], AF.Copy)
                nc.sync.dma_start(out=xT_loc_dram[:, b * P:(b + 1) * P],
                                  in_=xt[:])
                write_adloc(xt[:], wadn_sb, C, Hn, adlocn, b)

            def flush_3(b, pa):
                nvalid = NVALID_LAST if b == NBLK - 1 else P
                inv = sb.tile([P, 1], f32, tag="inv")
                nc.vector.tensor_scalar_add(out=inv[:], in0=pa[:, 32:33],
                                            scalar1=1e-16)
                nc.vector.reciprocal(out=inv[:], in_=inv[:])
                z = sb.tile([P, 32], f32, tag="zf")
                nc.vector.tensor_scalar_mul(out=z[:], in0=pa[:, 0:32],
                                            scalar1=inv[:])
                nc.vector.tensor_add(out=z[:], in0=z[:], in1=b3r_sb[:])
                nc.sync.dma_start(out=z_out[b * P:b * P + nvalid, :],
                                  in_=z[:nvalid, :])
                zt_ps = pssm.tile([32, P], f32, space="PSUM", tag="sm")
                nc.tensor.transpose(out=zt_ps[:], in_=z[:, :32],
                                    identity=idf_sb[:])
                zt = sb.tile([32, P], f32, tag="zt")
                nc.vector.tensor_copy(out=zt[:], in_=zt_ps[:])
                pm = pssm.tile([P, 32], f32, space="PSUM", tag="sm")
                nc.tensor.matmul(out=pm[:], lhsT=zt[:], rhs=wm_sb[:],
                                 start=True, stop=True)
                zm = sb.tile([P, 32], f32, tag="zm")
                nc.vector.tensor_add(out=zm[:], in0=pm[:], in1=bmr_sb[:])
                nc.sync.dma_start(out=zm_out[b * P:b * P + nvalid, :],
                                  in_=zm[:nvalid, :])
                pv = pssm.tile([P, 32], f32, space="PSUM", tag="sm")
                nc.tensor.matmul(out=pv[:], lhsT=zt[:], rhs=wv_sb[:],
                                 start=True, stop=True)
                zv = sb.tile([P, 32], f32, tag="zv")
                nc.vector.tensor_add(out=zv[:], in0=pv[:], in1=bvr_sb[:])
                nc.scalar.activation(zv[:], zv[:], AF.Exp)
                nc.vector.tensor_tensor(out=zv[:], in0=zv[:], in1=c100_sb[:],
                                        op=OP.min)
                nc.vector.tensor_tensor(out=zv[:], in0=zv[:], in1=c1em8_sb[:],
                                        op=OP.max)
                nc.sync.dma_start(out=zv_out[b * P:b * P + nvalid, :],
                                  in_=zv[:nvalid, :])

            # ================ the program ==================================
            rebuild(lambda t4: xT16[:, t4 * G * P:(t4 + 1) * G * P],
                    w1c_sb, 128, 260, tab1)
            for b in range(NBLK):
                write_adloc(xloc_sb[:, b * P:(b + 1) * P], wad1_sb, 128, 4,
                            adloc1, b)
            edge_layer(tab1, adloc1, 384, 4, 64,
                       lambda b, pa: flush_12(b, pa, 4, 64, b1r_sb, x2T_loc,
                                              wad2_sb, adloc2, 4))
            nc.gpsimd.collective_compute(
                "AllGather", mybir.AluOpType.bypass,
                replica_groups=[list(range(NCORES))],
                ins=[x2T_loc[:]], outs=[x2T_full[:]])
            rebuild(lambda t4: x2T_full[(G * t4) // NBLK, :,
                                        ((G * t4) % NBLK) * P:
                                        ((G * t4) % NBLK + G) * P],
                    w2c_sb, 64, 260, tab2)
            edge_layer(tab2, adloc2, 384, 4, 64,
                       lambda b, pa: flush_12(b, pa, 4, 64, b2r_sb, x3T_loc,
                                              wad3_sb, adloc3, 1))
            nc.gpsimd.collective_compute(
                "AllGather", mybir.AluOpType.bypass,
                replica_groups=[list(range(NCORES))],
                ins=[x3T_loc[:]], outs=[x3T_full[:]])
            rebuild(lambda t4: x3T_full[(G * t4) // NBLK, :,
                                        ((G * t4) % NBLK) * P:
                                        ((G * t4) % NBLK + G) * P],
                    w3c_sb, 64, 33, tab3)
            edge_layer(tab3, adloc3, 128, 1, 32, flush_3)

    if do_compile:
        nc.compile()
    return nc


def _make_in_maps(x, params, wrap_src, dstloc16, dlocrep, dlocprep,
                  Tmax):
    x = np.asarray(x, dtype=np.float32)

    def comb(W, a_s):
        W = np.asarray(W, np.float32)
        a_s = np.asarray(a_s, np.float32)
        heads, c = a_s.shape
        Wr = W.reshape(W.shape[0], heads, c)
        was = np.einsum('ihc,hc->ih', Wr, a_s)
        return np.concatenate([W, was], axis=1).astype(np.float16)

    def wadf(W, a_d):
        W = np.asarray(W, np.float32)
        a_d = np.asarray(a_d, np.float32)
        heads, c = a_d.shape
        Wr = W.reshape(W.shape[0], heads, c)
        return np.einsum('ihc,hc->ih', Wr, a_d).astype(np.float16)

    xT16 = np.zeros((P, NTOT), dtype=np.float16)
    for c in range(NCORES):
        xs = x[c * NPC:(c + 1) * NPC]
        xT16[:, c * NPAD:c * NPAD + NPC] = xs.T.astype(np.float16)

    def rep(v, n=P):
        v = np.asarray(v, np.float32).reshape(1, -1)
        return np.repeat(v, n, axis=0).astype(np.float32)

    common = dict(
        xT16=xT16,
        iota=np.tile(np.arange(P, dtype=np.float16), (P, 1)),
        iotac=np.arange(P, dtype=np.float16).reshape(P, 1),
        iotacr=np.tile(np.arange(P, dtype=np.float16).reshape(P, 1), (1, P)),
        iotabig=np.tile(np.arange(P, dtype=np.float16), (P, Tmax)),
        iotacrbig=np.tile(np.arange(P, dtype=np.float16).reshape(P, 1),
                          (1, Tmax * P)),
        c100=np.full((P, 32), 100.0, dtype=np.float32),
        c1em8=np.full((P, 32), 1e-8, dtype=np.float32),
        ident16=np.eye(P, dtype=np.float16),
        identf=np.eye(P, dtype=np.float32),
        w1c=comb(params['W1'], params['as1']),
        w2c=comb(params['W2'], params['as2']),
        w3c=comb(params['W3'], params['as3']),
        wad1=wadf(params['W1'], params['ad1']),
        wad2=wadf(params['W2'], params['ad2']),
        wad3=wadf(params['W3'], params['ad3']),
        b1r=rep(params['b1']), b2r=rep(params['b2']), b3r=rep(params['b3']),
        bmr=rep(params['bm']), bvr=rep(params['bv']),
        wm=np.asarray(params['Wm'], np.float32),
        wv=np.asarray(params['Wv'], np.float32),
    )
    in_maps = []
    for c in range(NCORES):
        m = dict(common)
        m.update(iwsrc=wrap_src[c], dloc=dstloc16[c], dlocrep=dlocrep[c],
                 dlocprep=dlocprep[c],
                 xlocT=xT16[:, c * NPAD:(c + 1) * NPAD].copy())
        in_maps.append(m)
    return in_maps


# ------------------------------------------------------------------ driver
def kernel(x, edge_index, W1, as1, ad1, b1, W2, as2, ad2, b2,
           W3, as3, ad3, b3, Wm, bm, Wv, bv):
    global LAST_RESULT
    import os
    from concourse.bass_utils import run_bass_kernel_spmd

    T, off8, offT, wrap_src, dstloc16, dlocrep, dlocprep = _preprocess(
        np.asarray(edge_index))
    params = dict(W1=W1, as1=as1, ad1=ad1, b1=b1, W2=W2, as2=as2, ad2=ad2,
                  b2=b2, W3=W3, as3=as3, ad3=ad3, b3=b3, Wm=Wm, bm=bm,
                  Wv=Wv, bv=bv)
    in_maps = _make_in_maps(x, params, wrap_src, dstloc16, dlocrep,
                            dlocprep, int(T.max()))

    nc = _build(T, off8, offT)
    res = run_bass_kernel_spmd(
        nc, in_maps, core_ids=list(range(NCORES)),
        trace=os.environ.get("BASS_TRACE", "") not in ("", "0"))
    LAST_RESULT = res

    z = np.concatenate([res.results[c]["z"] for c in range(NCORES)], axis=0)
    zm = np.concatenate([res.results[c]["zmean"] for c in range(NCORES)],
                        axis=0)
    zv = np.concatenate([res.results[c]["zvar"] for c in range(NCORES)],
                        axis=0)
    return zm, zv, z
